# revision 1
# baseline (speedup 1.0000x reference)
"""Distributed Trainium2 Bass kernel for nn_AttentionLayer_25993142075512.

Sharding: 8 cores = 2 batches x 4 head-groups (4 heads each). Each core
computes its batch's q/k/v projections for its 4 heads, causal attention,
and a partial output projection o @ Wo[head_rows]. Host sums the 4
partials per batch and adds bo. No on-device collectives.

Layout tricks:
  - All activations enter transposed (d on partitions): qT/kT/vT come out
    of weight-stationary matmuls directly.
  - RoPE: head dims are permuted host-side (evens @0:16, pass @16:32,
    odds @32:48, pass @48:64) for BOTH q and k (dot products invariant),
    so rotate_every_two becomes contiguous 16-partition block ops on DVE
    at legal partition bases (0/32/64/96).
  - Scores are computed transposed (kj on partitions) so softmaxed probs
    feed the o-matmul as the moving operand with kj as contraction dim.
  - Softmax denominator = ones column appended to v (row 64 of oT).
  - exp(0.125*s + causal_mask + kv_mask_bias) fused in one ACT op.
  - 1/denom applied to oT via a rank-1 broadcast matmul + DVE multiply.
  - q-blocks are processed in packs of 4 (512 cols) so scores/exp/oT ops
    amortize per-instruction overhead; PSUM accumulation uses one
    has_written group per 2KB zero-region (bank).

Assumes mask_q == 1 (spec fill=ones); mask_kv handled exactly.
"""

import sys, os, types, ctypes, contextlib

sys.path.insert(0, "/opt/trn_rl_repo")

import numpy as np
import ml_dtypes


def _install_axon_hooks():
    so = "/opt/axon/libaxon_pjrt.so"

    def _hook_factory(so_path):
        if not os.path.exists(so_path):
            return None
        lib = ctypes.CDLL(so_path)
        if not hasattr(lib, "axon_start_nrt_profile"):
            return None
        lib.axon_start_nrt_profile.argtypes = [
            ctypes.POINTER(ctypes.c_int64),
            ctypes.c_size_t,
        ]
        lib.axon_start_nrt_profile.restype = ctypes.c_int64
        lib.axon_stop_nrt_profile.argtypes = [ctypes.c_char_p]
        lib.axon_stop_nrt_profile.restype = ctypes.c_int64

        @contextlib.contextmanager
        def _hook(output_dir, device_ids):
            import jax

            jax.devices()
            if device_ids:
                ids = (ctypes.c_int64 * len(device_ids))(*device_ids)
                rc = lib.axon_start_nrt_profile(ids, len(device_ids))
            else:
                rc = lib.axon_start_nrt_profile(None, 0)
            if rc != 0:
                raise RuntimeError(f"axon_start_nrt_profile rc={rc}")
            try:
                yield
            finally:
                n = lib.axon_stop_nrt_profile(str(output_dir).encode())
                if n < 0:
                    raise RuntimeError(f"axon_stop_nrt_profile rc={n}")

        return _hook

    try:
        import antenv

        if "antenv.axon_hooks" not in sys.modules:
            hook = _hook_factory(so)
            mod = types.ModuleType("antenv.axon_hooks")
            mod.get_axon_ntff_profile_hook = lambda: hook
            mod.set_axon_ntff_profile_hook = lambda h: None
            antenv.axon_hooks = mod
            sys.modules["antenv.axon_hooks"] = mod
    except ImportError:
        pass
    from concourse import bass_utils

    bass_utils.upload_artifacts = lambda tmpdir: tmpdir


_install_axon_hooks()

from concourse import bass, bacc, tile, mybir  # noqa: E402

BF16 = mybir.dt.bfloat16
F32 = mybir.dt.float32
NPBF16 = ml_dtypes.bfloat16

B, N, DQ, DKV, H, DH, DOUT = 2, 2048, 1024, 1024, 16, 64, 1024
ROT = DH // 2  # 32
INF = 1.0e6
HPC = 4  # heads per core
NB = N // 128  # 16 q/k blocks
NG = NB // 4  # 4 q-block groups (packs of 4)
NQ4 = 4  # n quarters for projections
NQW = N // NQ4  # 512
VS = 72  # v' tile stride (64 v cols + ones col + pad)


def _head_perm():
    """Permute one head's 64 dims so RoPE even/odd blocks start at partition
    offsets 0 and 32: [evens(0,2..30), pass 32:48, odds(1,3..31), pass 48:64]."""
    ev = np.arange(0, ROT, 2)
    od = np.arange(1, ROT, 2)
    return np.concatenate([ev, np.arange(32, 48), od, np.arange(48, 64)])


def build_nc():
    _KP = int(os.environ.get("BASS_KERNEL_ABLATE", "5"))
    nc = bacc.Bacc(None, target_bir_lowering=False)

    sqt = nc.declare_dram_parameter("sqt", [DQ, N], BF16, isOutput=False)
    skvt = nc.declare_dram_parameter("skvt", [DKV, N], BF16, isOutput=False)
    wq = nc.declare_dram_parameter("wq", [8, 128, HPC * DH], BF16, isOutput=False)
    wkv = nc.declare_dram_parameter("wkv", [8, 128, HPC * 2 * DH], BF16, isOutput=False)
    wo = nc.declare_dram_parameter("wo", [2, 128, DOUT], BF16, isOutput=False)
    bq = nc.declare_dram_parameter("bq", [2, 128, 1], F32, isOutput=False)
    bkv = nc.declare_dram_parameter("bkv", [HPC, 128, 1], F32, isOutput=False)
    cost_d = nc.declare_dram_parameter("cost", [128, N], BF16, isOutput=False)
    sint_d = nc.declare_dram_parameter("sint", [128, N], BF16, isOutput=False)
    mtile_d = nc.declare_dram_parameter("mtile", [128, 128], BF16, isOutput=False)
    ident_d = nc.declare_dram_parameter("ident", [128, 128], BF16, isOutput=False)
    bmask_d = nc.declare_dram_parameter("bmask", [NB, 128, 1], F32, isOutput=False)
    out_ext = nc.declare_dram_parameter("out", [N, DOUT], BF16, isOutput=True)

    AF = mybir.ActivationFunctionType
    ALU = mybir.AluOpType

    with tile.TileContext(nc) as tc:
        with (
            tc.tile_pool(name="const", bufs=1) as cpool,
            tc.tile_pool(name="big", bufs=1) as bigpool,
            tc.tile_pool(name="stream", bufs=4) as spool,
            tc.tile_pool(name="ptile", bufs=8) as ppool,
            tc.tile_pool(name="small", bufs=8) as smallpool,
            tc.tile_pool(name="outsb", bufs=4) as outsb_pool,
        ):
            # ---- constants to SBUF ----
            wq_sb = []
            wkv_sb = []
            for c in range(8):
                t = cpool.tile([128, HPC * DH], BF16, tag=f"wq{c}", name=f"wq{c}")
                nc.sync.dma_start(t[:], wq[c])
                wq_sb.append(t)
                t2 = cpool.tile([128, HPC * 2 * DH], BF16, tag=f"wkv{c}", name=f"wkv{c}")
                nc.sync.dma_start(t2[:], wkv[c])
                wkv_sb.append(t2)
            bq_sb = cpool.tile([128, 2], F32, tag="bq", name="bq")
            for m in range(2):
                nc.sync.dma_start(bq_sb[:, m : m + 1], bq[m])
            bkv_sb = cpool.tile([128, HPC], F32, tag="bkv", name="bkv")
            for h in range(HPC):
                nc.sync.dma_start(bkv_sb[:, h : h + 1], bkv[h])
            # later-phase constants: tiles declared here, DMAs issued after the
            # projection stream so the first matmuls aren't queued behind them
            wo_sb = [cpool.tile([128, DOUT], BF16, tag=f"wo{pr}", name=f"wo{pr}") for pr in range(2)]
            cost = cpool.tile([128, N], BF16, tag="cost", name="cost")
            sint = cpool.tile([128, N], BF16, tag="sint", name="sint")
            mtile = cpool.tile([128, 128], BF16, tag="mtile", name="mtile")
            ident = cpool.tile([128, 128], BF16, tag="ident", name="ident")
            bmask = cpool.tile([128, NB], F32, tag="bmask", name="bmask")
            ones1 = cpool.tile([1, 64], BF16, tag="ones1", name="ones1")

            def _late_const_dmas():
                nc.sync.dma_start(cost[:], cost_d[:])
                nc.sync.dma_start(sint[:], sint_d[:])
                nc.sync.dma_start(mtile[:], mtile_d[:])
                nc.sync.dma_start(ident[:], ident_d[:])
                for kb in range(NB):
                    nc.sync.dma_start(bmask[:, kb : kb + 1], bmask_d[kb])
                for pr in range(2):
                    nc.sync.dma_start(wo_sb[pr][:], wo[pr])
                nc.vector.memset(ones1[:], 1.0)

            # ---- persistent activations ----
            qT = [bigpool.tile([128, N], BF16, tag=f"qT{i}", name=f"qT{i}") for i in range(2)]
            # kvT per head [128, N]: even head: kT rows 0:64, vT rows 64:128;
            # odd head: vT rows 0:64, kT rows 64:128 (parity-matched bases).
            kvT = [bigpool.tile([128, N], BF16, tag=f"kvT{h}", name=f"kvT{h}") for h in range(HPC)]
            # v' group tiles per (head, group): [128, 4, VS]; [:, j, 0:64] = v
            # for kb=4g+j, [:, j, 64] = ones (whole tile memset to 1 first).
            vg = [
                [bigpool.tile([128, 4, VS], BF16, tag=f"vg{h}_{g}", name=f"vg{h}_{g}") for g in range(NG)]
                for h in range(HPC)
            ]
            # normalized oT groups per (pair, group): [128, 512] bf16
            oTs = [
                [bigpool.tile([128, 512], BF16, tag=f"oTs{pr}_{g}", name=f"oTs{pr}_{g}") for g in range(NG)]
                for pr in range(2)
            ]

            def rope_block(dst, r0, c0=0, cw=N):
                """Rotary in-place: evens at dst[r0:r0+16], odds at
                dst[r0+32:r0+48], columns [c0, c0+cw)."""
                cs = slice(c0, c0 + cw)
                e = slice(r0, r0 + 16)
                o = slice(r0 + 32, r0 + 48)
                cE, sE = cost[e, cs], sint[e, cs]
                cO, sO = cost[o, cs], sint[o, cs]
                # plain tensor_tensor ops run in DVE 2x mode for bf16 (stt
                # has no fast mode). Sign of sin is baked into the host table:
                # odd-block rows hold -sin, even-block rows hold +sin.
                t1 = smallpool.tile([16, cw], BF16, tag="ropetmp1", name="ropetmp1", bufs=2)
                t2 = smallpool.tile([16, cw], BF16, tag="ropetmp2", name="ropetmp2", bufs=2)
                t3 = smallpool.tile([16, cw], BF16, tag="ropetmp3", name="ropetmp3", bufs=2)
                t4 = smallpool.tile([16, cw], BF16, tag="ropetmp4", name="ropetmp4", bufs=2)
                v = nc.vector
                v.tensor_mul(t1[:], dst[e, cs], cE)
                v.tensor_mul(t2[:], dst[o, cs], sO)
                v.tensor_mul(t3[:], dst[o, cs], cO)
                v.tensor_mul(t4[:], dst[e, cs], sE)
                v.tensor_add(dst[e, cs], t1[:], t2[:])
                v.tensor_add(dst[o, cs], t3[:], t4[:])

            # ================= phase 1: projections =================
            with tc.tile_pool(name="projpsum", bufs=1, space=bass.MemorySpace.PSUM) as pj:
                for nhf in range(2 if _KP >= 1 else 0):
                    h0_ = nhf * (N // 2)
                    xqs, xkvs = [], []
                    for c in range(8):
                        xq = spool.tile([128, N // 2], BF16, tag="xq", name="xq", bufs=10)
                        nc.sync.dma_start(xq[:], sqt[c * 128 : (c + 1) * 128, h0_ : h0_ + N // 2])
                        xkvt = spool.tile([128, N // 2], BF16, tag="xkv", name="xkv", bufs=10)
                        nc.sync.dma_start(xkvt[:], skvt[c * 128 : (c + 1) * 128, h0_ : h0_ + N // 2])
                        xqs.append(xq)
                        xkvs.append(xkvt)
                    if nhf == 0:
                        _late_const_dmas()
                    for sub in range(2):
                        nq0 = h0_ + sub * NQW
                        s0 = sub * NQW
                        ps_q = [pj.tile([128, NQW], F32, tag=f"psq{m}", name=f"psq{m}", bufs=2) for m in range(2)]
                        ps_kv = [pj.tile([128, NQW], F32, tag=f"pskv{h}", name=f"pskv{h}") for h in range(HPC)]
                        for c in range(8):
                            st = c == 0
                            sp = c == 7
                            for m in range(2):
                                nc.tensor.matmul(
                                    ps_q[m][:],
                                    wq_sb[c][:, m * 128 : (m + 1) * 128],
                                    xqs[c][:, s0 : s0 + NQW],
                                    start=st,
                                    stop=sp,
                                )
                            for h in range(HPC):
                                nc.tensor.matmul(
                                    ps_kv[h][:],
                                    wkv_sb[c][:, h * 128 : (h + 1) * 128],
                                    xkvs[c][:, s0 : s0 + NQW],
                                    start=st,
                                    stop=sp,
                                )
                        for m in range(2):
                            nc.scalar.activation(
                                qT[m][:, nq0 : nq0 + NQW],
                                ps_q[m][:],
                                AF.Identity,
                                bias=bq_sb[:, m : m + 1],
                            )
                        for h in range(HPC):
                            nc.scalar.activation(
                                kvT[h][:, nq0 : nq0 + NQW],
                                ps_kv[h][:],
                                AF.Identity,
                                bias=bkv_sb[:, h : h + 1],
                            )
                        if _KP >= 2:
                            for m in range(2):
                                rope_block(qT[m], 0, nq0, NQW)
                                rope_block(qT[m], 64, nq0, NQW)
                            for h in range(HPC):
                                rope_block(kvT[h], (h % 2) * 64, nq0, NQW)


            # ================= phase 2: v' build (transpose vT) =================
            with tc.tile_pool(name="vtpsum", bufs=2, space=bass.MemorySpace.PSUM) as vtp:
                for h in range(HPC if _KP >= 3 else 0):
                    vb = 64 if h % 2 == 0 else 0  # v rows base (host layout)
                    for g in range(NG):
                        nc.vector.memset(vg[h][g][:], 1.0)
                        pk = vtp.tile([128, 256], BF16, tag="vtp", name="vtp")
                        for j in range(4):
                            kb = 4 * g + j
                            nc.tensor.matmul(
                                pk[:, j * 64 : (j + 1) * 64],
                                kvT[h][vb : vb + 64, kb * 128 : (kb + 1) * 128],
                                ident[vb : vb + 64, vb : vb + 64],
                                is_transpose=True,
                                start=(j == 0),
                                stop=(j == 3),
                            )
                        nc.scalar.activation(vg[h][g][:, :, 0:64], pk[:], AF.Copy)

            # ================= phase 3: attention =================
            with (
                tc.tile_pool(name="stpsum", bufs=3, space=bass.MemorySpace.PSUM) as stp,
                tc.tile_pool(name="otpsum", bufs=1, space=bass.MemorySpace.PSUM) as otp,
            ):
                for h in range(HPC if _KP >= 4 else 0):
                    pr, hr = h // 2, (h % 2) * 64
                    kr = (h % 2) * 64  # k rows base (parity-matched to q slice)
                    oT = [otp.tile([65, 512], F32, tag=f"oT{g}", name=f"oT{g}") for g in range(NG)]
                    for kb in range(NB):
                        for g in range(kb // 4, NG):
                            q0 = max(kb, 4 * g)
                            off = (q0 % 4) * 128
                            w = (4 * g + 4 - q0) * 128
                            sTp = stp.tile([128, 512], F32, tag="sT", name="sT")
                            nc.tensor.matmul(
                                sTp[:, off : off + w],
                                kvT[h][kr : kr + 64, kb * 128 : (kb + 1) * 128],
                                qT[pr][hr : hr + 64, q0 * 128 : q0 * 128 + w],
                                start=True,
                                stop=True,
                            )
                            if q0 == kb:  # diagonal block: causal mask on DVE
                                nc.vector.tensor_add(
                                    sTp[:, off : off + 128],
                                    sTp[:, off : off + 128],
                                    mtile[:],
                                )
                            p = ppool.tile([128, 512], BF16, tag="p", name="p")
                            nc.scalar.activation(
                                p[:, off : off + w],
                                sTp[:, off : off + w],
                                AF.Exp,
                                bias=bmask[:, kb : kb + 1],
                                scale=0.125,
                            )
                            # one has_written group per PSUM bank: start zeroes
                            # the whole zero-region once (kb==0 writes the full
                            # 512 span); later partial spans overwrite stale
                            # slices on first touch, then accumulate.
                            nc.tensor.matmul(
                                oT[g][:, off : off + w],
                                vg[h][kb // 4][:, kb % 4, 0:65],
                                p[:, off : off + w],
                                start=(kb == 0),
                                stop=(kb == 4 * g + 3),
                            )
                    # normalize per group: oTs[pr][g][hr:hr+64] = oT[:64]/oT[64]
                    for g in range(NG):
                        rec = smallpool.tile([1, 512], F32, tag="rec", name="rec", bufs=2)
                        nc.vector.reciprocal(rec[:], oT[g][64:65, :])
                        recb = smallpool.tile([1, 512], BF16, tag="recb", name="recb", bufs=2)
                        nc.vector.tensor_copy(recb[:], rec[:])
                        bc = stp.tile([64, 512], F32, tag="bc", name="bc", bufs=1)
                        nc.tensor.matmul(bc[:], ones1[:], recb[:], start=True, stop=True)
                        bcs = smallpool.tile([64, 512], F32, tag="bcs", name="bcs", bufs=2)
                        nc.scalar.activation(bcs[:], bc[:], AF.Copy)
                        nc.vector.tensor_mul(
                            oTs[pr][g][hr : hr + 64, :],
                            oT[g][0:64, :],
                            bcs[:],
                        )

            # ================= phase 4: output projection =================
            with tc.tile_pool(name="outpsum", bufs=3, space=bass.MemorySpace.PSUM) as op:
                for qb in range(NB if _KP >= 5 else 0):
                    g, off = qb // 4, (qb % 4) * 128
                    po = op.tile([128, DOUT], F32, tag="po", name="po")
                    for pr in range(2):
                        for nh in range(2):
                            nc.tensor.matmul(
                                po[:, nh * 512 : (nh + 1) * 512],
                                oTs[pr][g][:, off : off + 128],
                                wo_sb[pr][:, nh * 512 : (nh + 1) * 512],
                                start=(pr == 0),
                                stop=(pr == 1),
                            )
                    ob = outsb_pool.tile([128, DOUT], BF16, tag="ob", name="ob")
                    if qb % 2 == 0:
                        nc.scalar.activation(ob[:], po[:], AF.Copy)
                    else:
                        nc.vector.tensor_copy(ob[:], po[:])
                    nc.sync.dma_start(out_ext[qb * 128 : (qb + 1) * 128, :], ob[:])

    nc.compile()
    return nc


def _prep_host(s_q, s_kv, mask_q, mask_kv, Wq, bq_, Wkv, bkv_, Wo, bo_):
    """Build per-core input maps (host-side shard + transform)."""
    perm = _head_perm()

    inv_freq = 1.0 / (10000.0 ** (np.arange(0, ROT, 2, dtype=np.float64) / ROT))
    t = np.arange(N, dtype=np.float64)[None, :] * inv_freq[:, None]  # [16, N]
    cosT = np.zeros((128, N), NPBF16)
    sinT = np.zeros((128, N), NPBF16)
    for rb in range(0, 128, 32):
        cosT[rb : rb + 16] = np.cos(t).astype(NPBF16)
        sgn = 1.0 if (rb // 32) % 2 == 0 else -1.0
        sinT[rb : rb + 16] = (sgn * np.sin(t)).astype(NPBF16)

    mt = np.zeros((128, 128), np.float32)
    pidx = np.arange(128)
    mt[pidx[:, None] > pidx[None, :]] = -INF
    mt = mt.astype(NPBF16)
    ident = np.eye(128, dtype=NPBF16)

    in_maps = []
    for core in range(8):
        b = core // 4
        h0 = (core % 4) * HPC

        wq_cols = []
        bq_cols = []
        for h in range(h0, h0 + HPC):
            cols = Wq[:, h * DH : (h + 1) * DH][:, perm]
            wq_cols.append(cols)
            bq_cols.append(bq_[h * DH : (h + 1) * DH][perm])
        wq_c = np.concatenate(wq_cols, axis=1)  # [1024, 256]
        bq_c = np.concatenate(bq_cols)  # [256]

        wkv_cols = []
        bkv_cols = []
        for h in range(h0, h0 + HPC):
            kcols = Wkv[:, h * 2 * DH : h * 2 * DH + DH][:, perm]
            vcols = Wkv[:, h * 2 * DH + DH : (h + 1) * 2 * DH]
            kb_ = bkv_[h * 2 * DH : h * 2 * DH + DH][perm]
            vb_ = bkv_[h * 2 * DH + DH : (h + 1) * 2 * DH]
            if (h - h0) % 2 == 0:  # even head: [k; v]
                wkv_cols.append(np.concatenate([kcols, vcols], axis=1))
                bkv_cols.append(np.concatenate([kb_, vb_]))
            else:  # odd head: [v; k] so k-rows sit at partition base 64
                wkv_cols.append(np.concatenate([vcols, kcols], axis=1))
                bkv_cols.append(np.concatenate([vb_, kb_]))
        wkv_c = np.concatenate(wkv_cols, axis=1)  # [1024, 512]

        wo_rows = Wo[h0 * DH : (h0 + HPC) * DH, :]  # [256, 1024]

        bmask = (INF * (mask_kv[b].astype(np.float32) - 1.0)).reshape(NB, 128, 1)

        in_maps.append(
            {
                "sqt": np.ascontiguousarray(s_q[b].T).astype(NPBF16),
                "skvt": np.ascontiguousarray(s_kv[b].T).astype(NPBF16),
                "wq": np.ascontiguousarray(wq_c.reshape(8, 128, HPC * DH)).astype(NPBF16),
                "wkv": np.ascontiguousarray(wkv_c.reshape(8, 128, HPC * 2 * DH)).astype(NPBF16),
                "wo": np.ascontiguousarray(wo_rows.reshape(2, 128, DOUT)).astype(NPBF16),
                "bq": bq_c.reshape(2, 128, 1).astype(np.float32),
                "bkv": np.stack(bkv_cols).reshape(HPC, 128, 1).astype(np.float32),
                "cost": cosT,
                "sint": sinT,
                "mtile": mt,
                "ident": ident,
                "bmask": bmask.astype(np.float32),
            }
        )
    return in_maps


_NC_CACHE = {}


def kernel(s_q, s_kv, mask_q, mask_kv, Wq, bq, Wkv, bkv, Wo, bo, _return_results=False):
    from concourse.bass_utils import run_bass_kernel_spmd

    if "nc" not in _NC_CACHE:
        _NC_CACHE["nc"] = build_nc()
    nc = _NC_CACHE["nc"]

    in_maps = _prep_host(
        np.asarray(s_q, np.float32),
        np.asarray(s_kv, np.float32),
        np.asarray(mask_q, np.float32),
        np.asarray(mask_kv, np.float32),
        np.asarray(Wq, np.float32),
        np.asarray(bq, np.float32),
        np.asarray(Wkv, np.float32),
        np.asarray(bkv, np.float32),
        np.asarray(Wo, np.float32),
        np.asarray(bo, np.float32),
    )
    trace = bool(int(os.environ.get("KERNEL_TRACE", "0")))
    res = run_bass_kernel_spmd(nc, in_maps, core_ids=list(range(8)), trace=trace)

    out = np.zeros((B, N, DOUT), np.float32)
    for core in range(8):
        b = core // 4
        out[b] += res.results[core]["out"].astype(np.float32)
    out += np.asarray(bo, np.float32)[None, None, :]
    if _return_results:
        return out, res
    return out



# revision 7
# speedup vs baseline: 1.1835x; 1.1835x over previous
"""Distributed Trainium2 Bass kernel for nn_AttentionLayer_25993142075512.

Sharding: 8 cores = 2 batches x 4 head-groups (4 heads each). Each core
computes its batch's q/k/v projections for its 4 heads, causal attention,
and a partial output projection o @ Wo[head_rows]. Host sums the 4
partials per batch and adds bo. No on-device collectives.

Layout tricks:
  - All activations enter transposed (d on partitions): qT/kT/vT come out
    of weight-stationary matmuls directly.
  - RoPE: head dims are permuted host-side (evens @0:16, pass @16:32,
    odds @32:48, pass @48:64) for BOTH q and k (dot products invariant),
    so rotate_every_two becomes contiguous 16-partition block ops on DVE
    at legal partition bases (0/32/64/96).
  - Scores are computed transposed (kj on partitions) so softmaxed probs
    feed the o-matmul as the moving operand with kj as contraction dim.
  - Softmax denominator = ones column appended to v (row 64 of oT).
  - exp(0.125*s + causal_mask + kv_mask_bias) fused in one ACT op.
  - 1/denom applied to oT via a rank-1 broadcast matmul + DVE multiply.
  - q-blocks are processed in packs of 4 (512 cols) so scores/exp/oT ops
    amortize per-instruction overhead; PSUM accumulation uses one
    has_written group per 2KB zero-region (bank).

Assumes mask_q == 1 (spec fill=ones); mask_kv handled exactly.
"""

import sys, os, types, ctypes, contextlib

sys.path.insert(0, "/opt/trn_rl_repo")

import numpy as np
import ml_dtypes


def _install_axon_hooks():
    so = "/opt/axon/libaxon_pjrt.so"

    def _hook_factory(so_path):
        if not os.path.exists(so_path):
            return None
        lib = ctypes.CDLL(so_path)
        if not hasattr(lib, "axon_start_nrt_profile"):
            return None
        lib.axon_start_nrt_profile.argtypes = [
            ctypes.POINTER(ctypes.c_int64),
            ctypes.c_size_t,
        ]
        lib.axon_start_nrt_profile.restype = ctypes.c_int64
        lib.axon_stop_nrt_profile.argtypes = [ctypes.c_char_p]
        lib.axon_stop_nrt_profile.restype = ctypes.c_int64

        @contextlib.contextmanager
        def _hook(output_dir, device_ids):
            import jax

            jax.devices()
            if device_ids:
                ids = (ctypes.c_int64 * len(device_ids))(*device_ids)
                rc = lib.axon_start_nrt_profile(ids, len(device_ids))
            else:
                rc = lib.axon_start_nrt_profile(None, 0)
            if rc != 0:
                raise RuntimeError(f"axon_start_nrt_profile rc={rc}")
            try:
                yield
            finally:
                n = lib.axon_stop_nrt_profile(str(output_dir).encode())
                if n < 0:
                    raise RuntimeError(f"axon_stop_nrt_profile rc={n}")

        return _hook

    try:
        import antenv

        if "antenv.axon_hooks" not in sys.modules:
            hook = _hook_factory(so)
            mod = types.ModuleType("antenv.axon_hooks")
            mod.get_axon_ntff_profile_hook = lambda: hook
            mod.set_axon_ntff_profile_hook = lambda h: None
            antenv.axon_hooks = mod
            sys.modules["antenv.axon_hooks"] = mod
    except ImportError:
        pass
    from concourse import bass_utils

    bass_utils.upload_artifacts = lambda tmpdir: tmpdir


_install_axon_hooks()

from concourse import bass, bacc, tile, mybir  # noqa: E402

BF16 = mybir.dt.bfloat16
F32 = mybir.dt.float32
NPBF16 = ml_dtypes.bfloat16

B, N, DQ, DKV, H, DH, DOUT = 2, 2048, 1024, 1024, 16, 64, 1024
ROT = DH // 2  # 32
INF = 1.0e6
HPC = 4  # heads per core
NB = N // 128  # 16 q/k blocks
NG = NB // 4  # 4 q-block groups (packs of 4)
NQ4 = 4  # n quarters for projections
NQW = N // NQ4  # 512
VS = 72  # v' tile stride (64 v cols + ones col + pad)


def _head_perm():
    """Permute one head's 64 dims so RoPE even/odd blocks start at partition
    offsets 0 and 32: [evens(0,2..30), pass 32:48, odds(1,3..31), pass 48:64]."""
    ev = np.arange(0, ROT, 2)
    od = np.arange(1, ROT, 2)
    return np.concatenate([ev, np.arange(32, 48), od, np.arange(48, 64)])


def build_nc():
    _KP = int(os.environ.get("BASS_KERNEL_ABLATE", "5"))
    nc = bacc.Bacc(None, target_bir_lowering=False)

    sqt = nc.declare_dram_parameter("sqt", [DQ, N], BF16, isOutput=False)
    skvt = nc.declare_dram_parameter("skvt", [DKV, N], BF16, isOutput=False)
    wq = nc.declare_dram_parameter("wq", [8, 128, HPC * DH], BF16, isOutput=False)
    wkv = nc.declare_dram_parameter("wkv", [8, 128, HPC * 2 * DH], BF16, isOutput=False)
    wo = nc.declare_dram_parameter("wo", [2, 128, DOUT], BF16, isOutput=False)
    bq = nc.declare_dram_parameter("bq", [2, 128, 1], F32, isOutput=False)
    bkv = nc.declare_dram_parameter("bkv", [HPC, 128, 1], F32, isOutput=False)
    cost_d = nc.declare_dram_parameter("cost", [128, N], BF16, isOutput=False)
    sint_d = nc.declare_dram_parameter("sint", [128, N], BF16, isOutput=False)
    mtile_d = nc.declare_dram_parameter("mtile", [128, 128], BF16, isOutput=False)
    ident_d = nc.declare_dram_parameter("ident", [128, 128], BF16, isOutput=False)
    bmask_d = nc.declare_dram_parameter("bmask", [NB, 128, 1], F32, isOutput=False)
    out_ext = nc.declare_dram_parameter("out", [N, DOUT], BF16, isOutput=True)

    AF = mybir.ActivationFunctionType
    ALU = mybir.AluOpType

    with tile.TileContext(nc) as tc:
        with (
            tc.tile_pool(name="const", bufs=1) as cpool,
            tc.tile_pool(name="big", bufs=1) as bigpool,
            tc.tile_pool(name="stream", bufs=4) as spool,
            tc.tile_pool(name="ptile", bufs=8) as ppool,
            tc.tile_pool(name="small", bufs=8) as smallpool,
            tc.tile_pool(name="outsb", bufs=4) as outsb_pool,
        ):
            # ---- constants to SBUF ----
            wq_sb = []
            wkv_sb = []
            for c in range(8):
                t = cpool.tile([128, HPC * DH], BF16, tag=f"wq{c}", name=f"wq{c}")
                nc.sync.dma_start(t[:], wq[c])
                wq_sb.append(t)
                t2 = cpool.tile([128, HPC * 2 * DH], BF16, tag=f"wkv{c}", name=f"wkv{c}")
                nc.sync.dma_start(t2[:], wkv[c])
                wkv_sb.append(t2)
            bq_sb = cpool.tile([128, 2], F32, tag="bq", name="bq")
            for m in range(2):
                nc.sync.dma_start(bq_sb[:, m : m + 1], bq[m])
            bkv_sb = cpool.tile([128, HPC], F32, tag="bkv", name="bkv")
            for h in range(HPC):
                nc.sync.dma_start(bkv_sb[:, h : h + 1], bkv[h])
            # later-phase constants: tiles declared here, DMAs issued after the
            # projection stream so the first matmuls aren't queued behind them
            wo_sb = [cpool.tile([128, DOUT], BF16, tag=f"wo{pr}", name=f"wo{pr}") for pr in range(2)]
            cost = cpool.tile([128, N], BF16, tag="cost", name="cost")
            sint = cpool.tile([128, N], BF16, tag="sint", name="sint")
            mtile = cpool.tile([128, 128], BF16, tag="mtile", name="mtile")
            ident = cpool.tile([128, 128], BF16, tag="ident", name="ident")
            bmask = cpool.tile([128, NB], F32, tag="bmask", name="bmask")
            ones1 = cpool.tile([1, 64], BF16, tag="ones1", name="ones1")

            def _late_const_dmas():
                nc.sync.dma_start(cost[:], cost_d[:])
                nc.sync.dma_start(sint[:], sint_d[:])
                nc.sync.dma_start(mtile[:], mtile_d[:])
                nc.sync.dma_start(ident[:], ident_d[:])
                for kb in range(NB):
                    nc.sync.dma_start(bmask[:, kb : kb + 1], bmask_d[kb])
                for pr in range(2):
                    nc.sync.dma_start(wo_sb[pr][:], wo[pr])
                nc.vector.memset(ones1[:], 1.0)

            # ---- persistent activations ----
            qT = [bigpool.tile([128, N], BF16, tag=f"qT{i}", name=f"qT{i}") for i in range(2)]
            # kvT per head [128, N]: even head: kT rows 0:64, vT rows 64:128;
            # odd head: vT rows 0:64, kT rows 64:128 (parity-matched bases).
            kvT = [bigpool.tile([128, N], BF16, tag=f"kvT{h}", name=f"kvT{h}") for h in range(HPC)]
            # v' group tiles per (head, group): [128, 4, VS]; [:, j, 0:64] = v
            # for kb=4g+j, [:, j, 64] = ones (whole tile memset to 1 first).
            vg = [
                [bigpool.tile([128, 4, VS], BF16, tag=f"vg{h}_{g}", name=f"vg{h}_{g}") for g in range(NG)]
                for h in range(HPC)
            ]
            # normalized oT groups per (pair, group): [128, 512] bf16
            oTs = [
                [bigpool.tile([128, 512], BF16, tag=f"oTs{pr}_{g}", name=f"oTs{pr}_{g}") for g in range(NG)]
                for pr in range(2)
            ]

            # hoist vg memsets to the head of the DVE queue
            for h in range(HPC):
                for g in range(NG):
                    nc.vector.memset(vg[h][g][:], 1.0)

            def rope_block(dst, r0, c0=0, cw=N):
                """Rotary in-place: evens at dst[r0:r0+16], odds at
                dst[r0+32:r0+48], columns [c0, c0+cw)."""
                cs = slice(c0, c0 + cw)
                e = slice(r0, r0 + 16)
                o = slice(r0 + 32, r0 + 48)
                cE, sE = cost[e, cs], sint[e, cs]
                cO, sO = cost[o, cs], sint[o, cs]
                # plain tensor_tensor ops run in DVE 2x mode for bf16 (stt
                # has no fast mode). Sign of sin is baked into the host table:
                # odd-block rows hold -sin, even-block rows hold +sin.
                t1 = smallpool.tile([16, cw], BF16, tag="ropetmp1", name="ropetmp1", bufs=2)
                t2 = smallpool.tile([16, cw], BF16, tag="ropetmp2", name="ropetmp2", bufs=2)
                t3 = smallpool.tile([16, cw], BF16, tag="ropetmp3", name="ropetmp3", bufs=2)
                t4 = smallpool.tile([16, cw], BF16, tag="ropetmp4", name="ropetmp4", bufs=2)
                v = nc.vector
                v.tensor_mul(t1[:], dst[e, cs], cE)
                v.tensor_mul(t2[:], dst[o, cs], sO)
                v.tensor_mul(t3[:], dst[o, cs], cO)
                v.tensor_mul(t4[:], dst[e, cs], sE)
                v.tensor_add(dst[e, cs], t1[:], t2[:])
                v.tensor_add(dst[o, cs], t3[:], t4[:])

            # ================= phase 1: projections =================
            with tc.tile_pool(name="projpsum", bufs=1, space=bass.MemorySpace.PSUM) as pj:
                for nhf in range(2 if _KP >= 1 else 0):
                    h0_ = nhf * (N // 2)
                    xqs, xkvs = [], []
                    for c in range(8):
                        xq = spool.tile([128, N // 2], BF16, tag="xq", name="xq", bufs=10)
                        nc.sync.dma_start(xq[:], sqt[c * 128 : (c + 1) * 128, h0_ : h0_ + N // 2])
                        xkvt = spool.tile([128, N // 2], BF16, tag="xkv", name="xkv", bufs=10)
                        nc.sync.dma_start(xkvt[:], skvt[c * 128 : (c + 1) * 128, h0_ : h0_ + N // 2])
                        xqs.append(xq)
                        xkvs.append(xkvt)
                    if nhf == 0:
                        _late_const_dmas()
                    for sub in range(2):
                        nq0 = h0_ + sub * NQW
                        s0 = sub * NQW
                        ps_q = [pj.tile([128, NQW], F32, tag=f"psq{m}", name=f"psq{m}", bufs=2) for m in range(2)]
                        ps_kv = [pj.tile([128, NQW], F32, tag=f"pskv{h}", name=f"pskv{h}") for h in range(HPC)]
                        for c in range(8):
                            st = c == 0
                            sp = c == 7
                            for m in range(2):
                                nc.tensor.matmul(
                                    ps_q[m][:],
                                    wq_sb[c][:, m * 128 : (m + 1) * 128],
                                    xqs[c][:, s0 : s0 + NQW],
                                    start=st,
                                    stop=sp,
                                )
                            for h in range(HPC):
                                nc.tensor.matmul(
                                    ps_kv[h][:],
                                    wkv_sb[c][:, h * 128 : (h + 1) * 128],
                                    xkvs[c][:, s0 : s0 + NQW],
                                    start=st,
                                    stop=sp,
                                )
                        for m in range(2):
                            nc.scalar.activation(
                                qT[m][:, nq0 : nq0 + NQW],
                                ps_q[m][:],
                                AF.Identity,
                                bias=bq_sb[:, m : m + 1],
                            )
                        for h in range(HPC):
                            nc.scalar.activation(
                                kvT[h][:, nq0 : nq0 + NQW],
                                ps_kv[h][:],
                                AF.Identity,
                                bias=bkv_sb[:, h : h + 1],
                            )
                        if _KP >= 2:
                            for m in range(2):
                                rope_block(qT[m], 0, nq0, NQW)
                                rope_block(qT[m], 64, nq0, NQW)
                            for h in range(HPC):
                                rope_block(kvT[h], (h % 2) * 64, nq0, NQW)


            # ================= phase 2: v' build (transpose vT) =================
            # memsets for vg were hoisted before phase 1 (emitted at the top of
            # the DVE queue) so phase 2/3 don't serialize behind the rope ops.
            with tc.tile_pool(name="vtpsum", bufs=2, space=bass.MemorySpace.PSUM) as vtp:
                for h in range(HPC if _KP >= 3 else 0):
                    vb = 64 if h % 2 == 0 else 0  # v rows base (host layout)
                    for g in range(NG):
                        pk = vtp.tile([128, 256], BF16, tag="vtp", name="vtp")
                        for j in range(4):
                            kb = 4 * g + j
                            nc.tensor.matmul(
                                pk[:, j * 64 : (j + 1) * 64],
                                kvT[h][vb : vb + 64, kb * 128 : (kb + 1) * 128],
                                ident[vb : vb + 64, vb : vb + 64],
                                is_transpose=True,
                                start=(j == 0),
                                stop=(j == 3),
                            )
                        nc.scalar.activation(vg[h][g][:, :, 0:64], pk[:], AF.Copy)

            # ================= phase 3: attention =================
            with (
                tc.tile_pool(name="stpsum", bufs=3, space=bass.MemorySpace.PSUM) as stp,
                tc.tile_pool(name="otpsum", bufs=1, space=bass.MemorySpace.PSUM) as otp,
            ):
                for h in range(HPC if _KP >= 4 else 0):
                    pr, hr = h // 2, (h % 2) * 64
                    kr = (h % 2) * 64  # k rows base (parity-matched to q slice)
                    oT = [otp.tile([65, 512], F32, tag=f"oT{g}", name=f"oT{g}") for g in range(NG)]
                    for kb in range(NB):
                        for g in range(kb // 4, NG):
                            q0 = max(kb, 4 * g)
                            off = (q0 % 4) * 128
                            w = (4 * g + 4 - q0) * 128
                            sTp = stp.tile([128, 512], F32, tag="sT", name="sT")
                            nc.tensor.matmul(
                                sTp[:, off : off + w],
                                kvT[h][kr : kr + 64, kb * 128 : (kb + 1) * 128],
                                qT[pr][hr : hr + 64, q0 * 128 : q0 * 128 + w],
                                start=True,
                                stop=True,
                            )
                            if q0 == kb:  # diagonal block: causal mask on DVE
                                nc.vector.tensor_add(
                                    sTp[:, off : off + 128],
                                    sTp[:, off : off + 128],
                                    mtile[:],
                                )
                            p = ppool.tile([128, 512], BF16, tag="p", name="p")
                            nc.scalar.activation(
                                p[:, off : off + w],
                                sTp[:, off : off + w],
                                AF.Exp,
                                bias=bmask[:, kb : kb + 1],
                                scale=0.125,
                            )
                            # one has_written group per PSUM bank: start zeroes
                            # the whole zero-region once (kb==0 writes the full
                            # 512 span); later partial spans overwrite stale
                            # slices on first touch, then accumulate.
                            nc.tensor.matmul(
                                oT[g][:, off : off + w],
                                vg[h][kb // 4][:, kb % 4, 0:65],
                                p[:, off : off + w],
                                start=(kb == 0),
                                stop=(kb == 4 * g + 3),
                            )
                    # normalize per group: oTs[pr][g][hr:hr+64] = oT[:64]/oT[64]
                    for g in range(NG):
                        rec = smallpool.tile([1, 512], F32, tag="rec", name="rec", bufs=2)
                        nc.vector.reciprocal(rec[:], oT[g][64:65, :])
                        recb = smallpool.tile([1, 512], BF16, tag="recb", name="recb", bufs=2)
                        nc.vector.tensor_copy(recb[:], rec[:])
                        bc = stp.tile([64, 512], F32, tag="bc", name="bc", bufs=1)
                        nc.tensor.matmul(bc[:], ones1[:], recb[:], start=True, stop=True)
                        bcs = smallpool.tile([64, 512], F32, tag="bcs", name="bcs", bufs=2)
                        nc.scalar.activation(bcs[:], bc[:], AF.Copy)
                        nc.vector.tensor_mul(
                            oTs[pr][g][hr : hr + 64, :],
                            oT[g][0:64, :],
                            bcs[:],
                        )

            # ================= phase 4: output projection =================
            with tc.tile_pool(name="outpsum", bufs=3, space=bass.MemorySpace.PSUM) as op:
                for qb in range(NB if _KP >= 5 else 0):
                    g, off = qb // 4, (qb % 4) * 128
                    po = op.tile([128, DOUT], F32, tag="po", name="po")
                    for pr in range(2):
                        for nh in range(2):
                            nc.tensor.matmul(
                                po[:, nh * 512 : (nh + 1) * 512],
                                oTs[pr][g][:, off : off + 128],
                                wo_sb[pr][:, nh * 512 : (nh + 1) * 512],
                                start=(pr == 0),
                                stop=(pr == 1),
                            )
                    ob = outsb_pool.tile([128, DOUT], BF16, tag="ob", name="ob")
                    if qb % 2 == 0:
                        nc.scalar.activation(ob[:], po[:], AF.Copy)
                    else:
                        nc.vector.tensor_copy(ob[:], po[:])
                    nc.sync.dma_start(out_ext[qb * 128 : (qb + 1) * 128, :], ob[:])

    nc.compile()
    return nc


def _prep_host(s_q, s_kv, mask_q, mask_kv, Wq, bq_, Wkv, bkv_, Wo, bo_):
    """Build per-core input maps (host-side shard + transform)."""
    perm = _head_perm()

    inv_freq = 1.0 / (10000.0 ** (np.arange(0, ROT, 2, dtype=np.float64) / ROT))
    t = np.arange(N, dtype=np.float64)[None, :] * inv_freq[:, None]  # [16, N]
    cosT = np.zeros((128, N), NPBF16)
    sinT = np.zeros((128, N), NPBF16)
    for rb in range(0, 128, 32):
        cosT[rb : rb + 16] = np.cos(t).astype(NPBF16)
        sgn = 1.0 if (rb // 32) % 2 == 0 else -1.0
        sinT[rb : rb + 16] = (sgn * np.sin(t)).astype(NPBF16)

    mt = np.zeros((128, 128), np.float32)
    pidx = np.arange(128)
    mt[pidx[:, None] > pidx[None, :]] = -INF
    mt = mt.astype(NPBF16)
    ident = np.eye(128, dtype=NPBF16)

    in_maps = []
    for core in range(8):
        b = core // 4
        h0 = (core % 4) * HPC

        wq_cols = []
        bq_cols = []
        for h in range(h0, h0 + HPC):
            cols = Wq[:, h * DH : (h + 1) * DH][:, perm]
            wq_cols.append(cols)
            bq_cols.append(bq_[h * DH : (h + 1) * DH][perm])
        wq_c = np.concatenate(wq_cols, axis=1)  # [1024, 256]
        bq_c = np.concatenate(bq_cols)  # [256]

        wkv_cols = []
        bkv_cols = []
        for h in range(h0, h0 + HPC):
            kcols = Wkv[:, h * 2 * DH : h * 2 * DH + DH][:, perm]
            vcols = Wkv[:, h * 2 * DH + DH : (h + 1) * 2 * DH]
            kb_ = bkv_[h * 2 * DH : h * 2 * DH + DH][perm]
            vb_ = bkv_[h * 2 * DH + DH : (h + 1) * 2 * DH]
            if (h - h0) % 2 == 0:  # even head: [k; v]
                wkv_cols.append(np.concatenate([kcols, vcols], axis=1))
                bkv_cols.append(np.concatenate([kb_, vb_]))
            else:  # odd head: [v; k] so k-rows sit at partition base 64
                wkv_cols.append(np.concatenate([vcols, kcols], axis=1))
                bkv_cols.append(np.concatenate([vb_, kb_]))
        wkv_c = np.concatenate(wkv_cols, axis=1)  # [1024, 512]

        wo_rows = Wo[h0 * DH : (h0 + HPC) * DH, :]  # [256, 1024]

        bmask = (INF * (mask_kv[b].astype(np.float32) - 1.0)).reshape(NB, 128, 1)

        in_maps.append(
            {
                "sqt": np.ascontiguousarray(s_q[b].T).astype(NPBF16),
                "skvt": np.ascontiguousarray(s_kv[b].T).astype(NPBF16),
                "wq": np.ascontiguousarray(wq_c.reshape(8, 128, HPC * DH)).astype(NPBF16),
                "wkv": np.ascontiguousarray(wkv_c.reshape(8, 128, HPC * 2 * DH)).astype(NPBF16),
                "wo": np.ascontiguousarray(wo_rows.reshape(2, 128, DOUT)).astype(NPBF16),
                "bq": bq_c.reshape(2, 128, 1).astype(np.float32),
                "bkv": np.stack(bkv_cols).reshape(HPC, 128, 1).astype(np.float32),
                "cost": cosT,
                "sint": sinT,
                "mtile": mt,
                "ident": ident,
                "bmask": bmask.astype(np.float32),
            }
        )
    return in_maps


_NC_CACHE = {}


def kernel(s_q, s_kv, mask_q, mask_kv, Wq, bq, Wkv, bkv, Wo, bo, _return_results=False):
    from concourse.bass_utils import run_bass_kernel_spmd

    if "nc" not in _NC_CACHE:
        _NC_CACHE["nc"] = build_nc()
    nc = _NC_CACHE["nc"]

    in_maps = _prep_host(
        np.asarray(s_q, np.float32),
        np.asarray(s_kv, np.float32),
        np.asarray(mask_q, np.float32),
        np.asarray(mask_kv, np.float32),
        np.asarray(Wq, np.float32),
        np.asarray(bq, np.float32),
        np.asarray(Wkv, np.float32),
        np.asarray(bkv, np.float32),
        np.asarray(Wo, np.float32),
        np.asarray(bo, np.float32),
    )
    trace = bool(int(os.environ.get("KERNEL_TRACE", "0")))
    res = run_bass_kernel_spmd(nc, in_maps, core_ids=list(range(8)), trace=trace)

    out = np.zeros((B, N, DOUT), np.float32)
    for core in range(8):
        b = core // 4
        out[b] += res.results[core]["out"].astype(np.float32)
    out += np.asarray(bo, np.float32)[None, None, :]
    if _return_results:
        return out, res
    return out



# revision 30
# speedup vs baseline: 1.3544x; 1.1444x over previous
"""Distributed Trainium2 Bass kernel for nn_AttentionLayer_25993142075512.

Sharding: 8 cores = 2 batches x 4 head-groups (4 heads each). Each core
computes its batch's q/k/v projections for its 4 heads, causal attention,
and a partial output projection o @ Wo[head_rows]. Host sums the 4
partials per batch and adds bo. No on-device collectives.

v2 design notes:
  - Head dims permuted host-side to [evens(16), odds(16), pass(32)] for both
    q and k (dot products invariant), so RoPE is 4 contiguous-block DVE ops
    per head per 1024-col chunk.
  - qT[pr]/kT[pr] hold a HEAD PAIR: head 2pr at partitions 0:64, head 2pr+1
    at 64:128. Score matmuls for the two heads go to PE row tiles (0,0) and
    (64,0) and run CONCURRENTLY (2x row tiling).
  - vT produced directly in [keys, dims] layout by a second projection pass
    (stationary = s_kv^T chunk, moving = Wv columns) - no transpose phase.
    v bias is folded into bo on the host (o += bv*denom trick).
  - oT matmuls are K-split into two 64-key halves on row tiles T0/T8 so the
    ENTIRE attention phase stays in (64,128) PE tiling mode - no mode-switch
    drains. Softmax denominator comes free via a ones column (M=65).
  - exp for both heads fused in one ACT op [128, 2, w] reading two psum banks.
  - Normalize: denominators collected to a [16,512] tile, one reciprocal,
    broadcast via one-hot E matmuls (still 64-mode), DVE multiply.
  - Projections per 512-col sub interleaved with attention groups of pair 0.

Assumes mask_q == 1 (spec fill=ones); mask_kv handled exactly via exp bias.
"""

import sys, os, types, ctypes, contextlib

sys.path.insert(0, "/opt/trn_rl_repo")

import numpy as np
import ml_dtypes


def _install_axon_hooks():
    so = "/opt/axon/libaxon_pjrt.so"

    def _hook_factory(so_path):
        if not os.path.exists(so_path):
            return None
        lib = ctypes.CDLL(so_path)
        if not hasattr(lib, "axon_start_nrt_profile"):
            return None
        lib.axon_start_nrt_profile.argtypes = [
            ctypes.POINTER(ctypes.c_int64),
            ctypes.c_size_t,
        ]
        lib.axon_start_nrt_profile.restype = ctypes.c_int64
        lib.axon_stop_nrt_profile.argtypes = [ctypes.c_char_p]
        lib.axon_stop_nrt_profile.restype = ctypes.c_int64

        @contextlib.contextmanager
        def _hook(output_dir, device_ids):
            import jax

            jax.devices()
            if device_ids:
                ids = (ctypes.c_int64 * len(device_ids))(*device_ids)
                rc = lib.axon_start_nrt_profile(ids, len(device_ids))
            else:
                rc = lib.axon_start_nrt_profile(None, 0)
            if rc != 0:
                raise RuntimeError(f"axon_start_nrt_profile rc={rc}")
            try:
                yield
            finally:
                n = lib.axon_stop_nrt_profile(str(output_dir).encode())
                if n < 0:
                    raise RuntimeError(f"axon_stop_nrt_profile rc={n}")

        return _hook

    try:
        import antenv

        if "antenv.axon_hooks" not in sys.modules:
            hook = _hook_factory(so)
            mod = types.ModuleType("antenv.axon_hooks")
            mod.get_axon_ntff_profile_hook = lambda: hook
            mod.set_axon_ntff_profile_hook = lambda h: None
            antenv.axon_hooks = mod
            sys.modules["antenv.axon_hooks"] = mod
    except ImportError:
        pass
    from concourse import bass_utils

    bass_utils.upload_artifacts = lambda tmpdir: tmpdir


_install_axon_hooks()

from concourse import bass, bacc, tile, mybir  # noqa: E402

BF16 = mybir.dt.bfloat16
F32 = mybir.dt.float32
NPBF16 = ml_dtypes.bfloat16

B, N, DQ, DKV, H, DH, DOUT = 2, 2048, 1024, 1024, 16, 64, 1024
ROT = DH // 2  # 32
INF = 1.0e6
HPC = 4  # heads per core (2 pairs)
NB = N // 128  # 16 k-blocks
NG = NB // 4  # 4 q-block groups (512 cols each)
NS = 4  # projection subs (512 cols each)
VS = 66  # vg per-(kb,pr,hh) stride (64 v cols + ones col + pad)


def build_nc():
    nc = bacc.Bacc(None, target_bir_lowering=False)

    sqt_d = nc.declare_dram_parameter("sqt", [DQ, N], BF16, isOutput=False)
    skvt_d = nc.declare_dram_parameter("skvt", [DKV, N], BF16, isOutput=False)
    wq_d = nc.declare_dram_parameter("wq", [2, 8, 128, 128], BF16, isOutput=False)
    wk_d = nc.declare_dram_parameter("wk", [2, 8, 128, 128], BF16, isOutput=False)
    wq2_d = nc.declare_dram_parameter("wq2", [8, 128, 128], BF16, isOutput=False)
    wk2_d = nc.declare_dram_parameter("wk2", [8, 128, 128], BF16, isOutput=False)
    bq2_d = nc.declare_dram_parameter("bq2", [128, 1], F32, isOutput=False)
    bk2_d = nc.declare_dram_parameter("bk2", [128, 1], F32, isOutput=False)
    wv_d = nc.declare_dram_parameter("wv", [8, 128, 256], BF16, isOutput=False)
    wo_d = nc.declare_dram_parameter("wo", [2, 128, DOUT], BF16, isOutput=False)
    bq_d = nc.declare_dram_parameter("bq", [2, 128, 1], F32, isOutput=False)
    bk_d = nc.declare_dram_parameter("bk", [2, 128, 1], F32, isOutput=False)
    cost_d = nc.declare_dram_parameter("cost", [128, N], BF16, isOutput=False)
    sint_d = nc.declare_dram_parameter("sint", [128, N], BF16, isOutput=False)
    mtile_d = nc.declare_dram_parameter("mtile", [128, 2, 128], BF16, isOutput=False)
    e64_d = nc.declare_dram_parameter("e64", [8, 64, 128], BF16, isOutput=False)
    bmask_d = nc.declare_dram_parameter("bmask", [NB, 128, 1], F32, isOutput=False)
    out_ext = nc.declare_dram_parameter("out", [N, DOUT], BF16, isOutput=True)

    AF = mybir.ActivationFunctionType
    ALU = mybir.AluOpType

    with tile.TileContext(nc) as tc:
        with (
            tc.tile_pool(name="const", bufs=1) as cpool,
            tc.tile_pool(name="big", bufs=1) as bigpool,
            tc.tile_pool(name="small", bufs=8) as smallpool,
            tc.tile_pool(name="ptile", bufs=4) as ppool,
            tc.tile_pool(name="outsb", bufs=4) as outsb_pool,
        ):
            # ---- SBUF constants ----
            wq_sb = [cpool.tile([128, 8 * 128], BF16, tag=f"wq{p}", name=f"wq{p}") for p in range(2)]
            wk_sb = [cpool.tile([128, 8 * 128], BF16, tag=f"wk{p}", name=f"wk{p}") for p in range(2)]
            wq2_sb = cpool.tile([128, 8 * 128], BF16, tag="wq2", name="wq2")
            wk2_sb = cpool.tile([128, 8 * 128], BF16, tag="wk2", name="wk2")
            bq2_sb = cpool.tile([128, 1], F32, tag="bq2", name="bq2")
            bk2_sb = cpool.tile([128, 1], F32, tag="bk2", name="bk2")
            wv_sb = cpool.tile([128, 8, 256], BF16, tag="wv", name="wv")
            wo_sb = [cpool.tile([128, DOUT], BF16, tag=f"wo{p}", name=f"wo{p}") for p in range(2)]
            bq_sb = cpool.tile([128, 2], F32, tag="bq", name="bq")
            bk_sb = cpool.tile([128, 2], F32, tag="bk", name="bk")
            cost = cpool.tile([128, N], BF16, tag="cost", name="cost")
            sint = cpool.tile([128, N], BF16, tag="sint", name="sint")
            mtile = cpool.tile([128, 2, 128], BF16, tag="mtile", name="mtile")
            e64 = [cpool.tile([64, 128], BF16, tag=f"e64_{i}", name=f"e64_{i}") for i in range(8)]
            bmask = cpool.tile([128, NB], F32, tag="bmask", name="bmask")

            # full-resident transposed inputs, 8 chunks of 128 dq-dims each
            sqt = [bigpool.tile([128, N], BF16, tag=f"sqt{c}", name=f"sqt{c}") for c in range(8)]
            skvt = [bigpool.tile([128, N], BF16, tag=f"skvt{c}", name=f"skvt{c}") for c in range(8)]

            # early-phase constant DMAs
            for p in range(2):
                for c in range(8):
                    nc.sync.dma_start(wq_sb[p][:, c * 128 : (c + 1) * 128], wq_d[p, c])
                    nc.sync.dma_start(wk_sb[p][:, c * 128 : (c + 1) * 128], wk_d[p, c])
            for c in range(8):
                nc.sync.dma_start(wq2_sb[:, c * 128 : (c + 1) * 128], wq2_d[c])
                nc.sync.dma_start(wk2_sb[:, c * 128 : (c + 1) * 128], wk2_d[c])
            nc.sync.dma_start(bq2_sb[:], bq2_d[:])
            nc.sync.dma_start(bk2_sb[:], bk2_d[:])
            for c in range(8):
                nc.sync.dma_start(wv_sb[:, c, :], wv_d[c])
            for p in range(2):
                nc.sync.dma_start(bq_sb[:, p : p + 1], bq_d[p])
                nc.sync.dma_start(bk_sb[:, p : p + 1], bk_d[p])
            nc.sync.dma_start(cost[:], cost_d[:])
            nc.sync.dma_start(sint[:], sint_d[:])
            # input DMAs per (chunk, sub) so projections start early
            for s in range(NS):
                for c in range(8):
                    cs = slice(s * 512, (s + 1) * 512)
                    nc.sync.dma_start(sqt[c][:, cs], sqt_d[c * 128 : (c + 1) * 128, cs])
                    nc.sync.dma_start(skvt[c][:, cs], skvt_d[c * 128 : (c + 1) * 128, cs])

            def _late_const_dmas():
                nc.sync.dma_start(mtile[:], mtile_d[:])
                for i in range(8):
                    nc.sync.dma_start(e64[i][:], e64_d[i])
                for kb in range(NB):
                    nc.sync.dma_start(bmask[:, kb : kb + 1], bmask_d[kb])
                for p in range(2):
                    nc.sync.dma_start(wo_sb[p][:], wo_d[p])

            # ---- persistent activations ----
            qT = [bigpool.tile([128, N], BF16, tag=f"qT{p}", name=f"qT{p}") for p in range(2)]
            kT = [bigpool.tile([128, N], BF16, tag=f"kT{p}", name=f"kT{p}") for p in range(2)]
            # rotate_every_two(q_full) copies: head (pr,hh) rot rows at
            # partition 64*pr + 32*hh (all 32-aligned)
            q2all = bigpool.tile([128, N], BF16, tag="q2all", name="q2all")
            k2all = bigpool.tile([128, N], BF16, tag="k2all", name="k2all")
            # vgAll[:, kb, pr, hh, 0:64] = v of head 2pr+hh for key block kb,
            # [:, kb, pr, hh, 64] = ones (denominator column)
            vgAll = bigpool.tile([128, NB, 2, 2, VS], BF16, tag="vg", name="vg")
            # normalized o^T per (pair, group): rows = 2 heads x 64 v-dims
            oTs = [
                [bigpool.tile([128, 512], BF16, tag=f"oTs{p}_{g}", name=f"oTs{p}_{g}") for g in range(NG)]
                for p in range(2)
            ]
            # denominator sums on partition 0 (free-dim indexed: alignment rule
            # forbids engine writes at partition bases not 0 mod 32); a
            # SBUF->SBUF DMA later spreads them across 16 partitions.
            dsum = bigpool.tile([1, 16, 512], F32, tag="dsum", name="dsum")
            denoms = bigpool.tile([16, 512], F32, tag="denoms", name="denoms")
            rec = bigpool.tile([16, 512], F32, tag="rec", name="rec")
            recb = bigpool.tile([64, 512], BF16, tag="recb", name="recb")

            # hoisted memsets (head of DVE queue): ones cols + recb zero-fill
            nc.vector.memset(vgAll[:], 1.0)
            nc.vector.memset(recb[:], 0.0)

            def rope_block(dst, dst2, pr, hh, c0, cw):
                """RoPE in place on dst[64*hh : 64*hh+32, c0:c0+cw] using the
                pre-swapped copy dst2 (rows 64*pr+32*hh): out = q*cos + q2*sin.
                All partition bases are 0 mod 32."""
                cs = slice(c0, c0 + cw)
                r = 64 * hh
                r2 = 64 * pr + 32 * hh
                t1 = smallpool.tile([32, cw], BF16, tag="ropet1", name="ropet1", bufs=2)
                t2 = smallpool.tile([32, cw], BF16, tag="ropet2", name="ropet2", bufs=2)
                v = nc.vector
                v.tensor_mul(t2[:, :], dst2[r2 : r2 + 32, cs], sint[r2 : r2 + 32, cs])
                v.tensor_mul(t1[:, :], dst[r : r + 32, cs], cost[r : r + 32, cs])
                v.tensor_add(dst[r : r + 32, cs], t1[:, :], t2[:, :])

            # ---- attention helpers ----
            def attn_group(pr, g, stq, otq):
                """Emit attention for head pair pr, query group g (cols
                g*512:(g+1)*512). All matmuls in (64,128) tiling mode."""
                oT = {}
                for hh in range(2):
                    for half in range(2):
                        oT[(hh, half)] = otq.tile(
                            [128, 512], F32, tag=f"oT{hh}{half}", name=f"oT{hh}{half}"
                        )
                for kb in range(4 * g + 4):
                    q0 = max(kb, 4 * g)
                    off = (q0 % 4) * 128
                    w = 512 - off
                    qs = slice(g * 512 + off, (g + 1) * 512)
                    ks = slice(kb * 128, (kb + 1) * 128)
                    sT = stq.tile([128, 2, 512], F32, tag="sT", name="sT")
                    nc.tensor.matmul(
                        sT[:, 0, off:], kT[pr][0:64, ks], qT[pr][0:64, qs],
                        start=True, stop=True,
                    )
                    nc.tensor.matmul(
                        sT[:, 1, off:], kT[pr][64:128, ks], qT[pr][64:128, qs],
                        start=True, stop=True,
                    )
                    if q0 == kb:  # diagonal block: causal mask, both heads
                        nc.vector.tensor_add(
                            sT[:, :, off : off + 128], sT[:, :, off : off + 128], mtile[:]
                        )
                    p = ppool.tile([128, 2, 512], BF16, tag="p", name="p")
                    nc.scalar.activation(
                        p[:, :, off:], sT[:, :, off:], AF.Exp,
                        bias=bmask[:, kb : kb + 1], scale=0.125,
                    )
                    st = kb == 0
                    sp = kb == 4 * g + 3
                    j = kb  # key block index into vgAll
                    # 4 K=64 half-matmuls on alternating row tiles; pairs
                    # (h0a,T0)//(h1b,T8) then (h0b,T8)//(h1a,T0) run concurrently.
                    nc.tensor.matmul(
                        oT[(0, 0)][0:65, off:], vgAll[0:64, j, pr, 0, 0:65],
                        p[0:64, 0, off:], start=st, stop=sp,
                    )
                    nc.tensor.matmul(
                        oT[(1, 1)][0:65, off:], vgAll[64:128, j, pr, 1, 0:65],
                        p[64:128, 1, off:], start=st, stop=sp,
                    )
                    nc.tensor.matmul(
                        oT[(0, 1)][0:65, off:], vgAll[64:128, j, pr, 0, 0:65],
                        p[64:128, 0, off:], start=st, stop=sp,
                    )
                    nc.tensor.matmul(
                        oT[(1, 0)][0:65, off:], vgAll[0:64, j, pr, 1, 0:65],
                        p[0:64, 1, off:], start=st, stop=sp,
                    )
                # evacuate unnormalized o and denominators
                for hh in range(2):
                    a, b = oT[(hh, 0)], oT[(hh, 1)]
                    tmp = smallpool.tile([65, 512], F32, tag="tmp65", name="tmp65", bufs=2)
                    nc.scalar.activation(tmp[:], a[0:65, :], AF.Copy)
                    nc.vector.tensor_add(
                        oTs[pr][g][hh * 64 : hh * 64 + 64, :], tmp[0:64, :], b[0:64, :]
                    )
                    idx = pr * 8 + g * 2 + hh
                    nc.vector.tensor_add(
                        dsum[0:1, idx, :], tmp[64:65, :], b[64:65, :]
                    )

            # ================= phase 1+2: projections + attention =================
            with (
                tc.tile_pool(name="projps", bufs=6, space=bass.MemorySpace.PSUM) as pj,
                tc.tile_pool(name="vtps", bufs=2, space=bass.MemorySpace.PSUM) as vtp,
            ):
                for s in range(NS):
                    cs = slice(s * 512, (s + 1) * 512)
                    if s == 0:
                        _late_const_dmas()
                    # q/k projections for both pairs + rotated copies q2/k2
                    projs = [
                        (wq_sb[0], bq_sb[:, 0:1], qT[0], sqt),
                        (wq_sb[1], bq_sb[:, 1:2], qT[1], sqt),
                        (wq2_sb, bq2_sb[:], q2all, sqt),
                        (wk_sb[0], bk_sb[:, 0:1], kT[0], skvt),
                        (wk_sb[1], bk_sb[:, 1:2], kT[1], skvt),
                        (wk2_sb, bk2_sb[:], k2all, skvt),
                    ]
                    for wsb, bsb, dst, src in projs:
                        ps = pj.tile([128, 512], F32, tag="pj", name="pj")
                        for c in range(8):
                            nc.tensor.matmul(
                                ps[:],
                                wsb[:, c * 128 : (c + 1) * 128],
                                src[c][:, cs],
                                start=(c == 0), stop=(c == 7),
                            )
                        nc.vector.tensor_scalar(dst[:, cs], ps[:], bsb, None, ALU.add)
                    # vT projection for the 4 key blocks of this sub
                    for j in range(4):
                        kb = 4 * s + j
                        ks = slice(kb * 128, (kb + 1) * 128)
                        pv = vtp.tile([128, 256], F32, tag="pv", name="pv")
                        for c in range(8):
                            nc.tensor.matmul(
                                pv[:], skvt[c][:, ks], wv_sb[:, c, :],
                                start=(c == 0), stop=(c == 7),
                            )
                        nc.vector.tensor_copy(vgAll[:, kb, :, :, 0:64], pv[:])
                    # rope on 1024-col chunks (every second sub)
                    if s % 2 == 1:
                        c0 = (s - 1) * 512
                        for p in range(2):
                            for hh in range(2):
                                rope_block(qT[p], q2all, p, hh, c0, 1024)
                                rope_block(kT[p], k2all, p, hh, c0, 1024)

            # ================= phase 3: attention =================
            with (
                tc.tile_pool(name="stps", bufs=2, space=bass.MemorySpace.PSUM) as stq,
                tc.tile_pool(name="otps", bufs=1, space=bass.MemorySpace.PSUM) as otq,
            ):
                for pr in range(2):
                    for g in range(NG):
                        attn_group(pr, g, stq, otq)

            # ================= phase 4: normalize + output projection =================
            with tc.tile_pool(name="outps", bufs=3, space=bass.MemorySpace.PSUM) as op:
                # spread the 16 denominator rows across partitions via DMA,
                # then one reciprocal, broadcast matmuls (64-row mode) + mul.
                nc.sync.dma_start(denoms[:], dsum[0:1, :, :])
                nc.vector.reciprocal(rec[:], denoms[:])
                nc.vector.tensor_copy(recb[0:16, :], rec[:])
                for pr in range(2):
                    for g in range(NG):
                        bc = op.tile([128, 512], F32, tag="bc", name="bc", bufs=2)
                        nc.tensor.matmul(
                            bc[:], e64[pr * 4 + g][:], recb[:], start=True, stop=True,
                            tile_position=(0, 0),
                        )
                        nc.vector.tensor_mul(oTs[pr][g][:], oTs[pr][g][:], bc[:])
                for qb in range(NB):
                    g, off = qb // 4, (qb % 4) * 128
                    po = op.tile([128, DOUT], F32, tag="po", name="po")
                    for p in range(2):
                        for nh in range(2):
                            nc.tensor.matmul(
                                po[:, nh * 512 : (nh + 1) * 512],
                                oTs[p][g][:, off : off + 128],
                                wo_sb[p][:, nh * 512 : (nh + 1) * 512],
                                start=(p == 0), stop=(p == 1),
                            )
                    ob = outsb_pool.tile([128, DOUT], BF16, tag="ob", name="ob")
                    if qb % 2 == 0:
                        nc.scalar.activation(ob[:], po[:], AF.Copy)
                    else:
                        nc.vector.tensor_copy(ob[:], po[:])
                    nc.sync.dma_start(out_ext[qb * 128 : (qb + 1) * 128, :], ob[:])

    nc.compile()
    return nc


def _rot2(cols):
    """rotate_every_two on the column axis of a [*, 64] block: only the first
    ROT dims rotate; returns the 32 rotated columns [- c1, c0, -c3, c2, ...]."""
    out = np.zeros_like(cols[:, :ROT])
    out[:, 0::2] = -cols[:, 1:ROT:2]
    out[:, 1::2] = cols[:, 0:ROT:2]
    return out


def _prep_host(s_q, s_kv, mask_q, mask_kv, Wq, bq_, Wkv, bkv_, Wo, bo_):
    inv_freq = 1.0 / (10000.0 ** (np.arange(0, ROT, 2, dtype=np.float64) / ROT))
    t = np.arange(N, dtype=np.float64)[None, :] * inv_freq[:, None]  # [16, N]
    cos32 = np.repeat(np.cos(t), 2, axis=0).astype(NPBF16)  # [32, N]
    sin32 = np.repeat(np.sin(t), 2, axis=0).astype(NPBF16)
    cosT = np.zeros((128, N), NPBF16)
    sinT = np.zeros((128, N), NPBF16)
    for r in (0, 64):
        cosT[r : r + 32] = cos32
    for r in (0, 32, 64, 96):
        sinT[r : r + 32] = sin32

    mt = np.zeros((128, 128), np.float32)
    pidx = np.arange(128)
    mt[pidx[:, None] > pidx[None, :]] = -INF
    mtile2 = np.stack([mt, mt], axis=1).astype(NPBF16)  # [128, 2, 128]

    e64 = np.zeros((8, 64, 128), NPBF16)
    for pr in range(2):
        for g in range(NG):
            e64[pr * 4 + g, pr * 8 + g * 2 + 0, 0:64] = 1.0
            e64[pr * 4 + g, pr * 8 + g * 2 + 1, 64:128] = 1.0

    in_maps = []
    for core in range(8):
        b = core // 4
        h0 = (core % 4) * HPC

        wq = np.zeros((2, 8, 128, 128), NPBF16)
        wk = np.zeros((2, 8, 128, 128), NPBF16)
        bqp = np.zeros((2, 128, 1), np.float32)
        bkp = np.zeros((2, 128, 1), np.float32)
        wq2 = np.zeros((1024, 128), np.float32)
        wk2 = np.zeros((1024, 128), np.float32)
        bq2 = np.zeros((128, 1), np.float32)
        bk2 = np.zeros((128, 1), np.float32)
        for pr in range(2):
            cols_q, cols_k, bq_c, bk_c = [], [], [], []
            for hh in range(2):
                h = h0 + 2 * pr + hh
                qcols = Wq[:, h * DH : (h + 1) * DH]
                kcols = Wkv[:, h * 2 * DH : h * 2 * DH + DH]
                cols_q.append(qcols)
                bq_c.append(bq_[h * DH : (h + 1) * DH])
                cols_k.append(kcols)
                bk_c.append(bkv_[h * 2 * DH : h * 2 * DH + DH])
                r2 = 64 * pr + 32 * hh
                wq2[:, r2 : r2 + 32] = _rot2(qcols)
                wk2[:, r2 : r2 + 32] = _rot2(kcols)
                bq2[r2 : r2 + 32, 0] = _rot2(bq_[h * DH : (h + 1) * DH][None, :])[0]
                bk2[r2 : r2 + 32, 0] = _rot2(
                    bkv_[h * 2 * DH : h * 2 * DH + DH][None, :]
                )[0]
            wq[pr] = np.concatenate(cols_q, axis=1).reshape(8, 128, 128).astype(NPBF16)
            wk[pr] = np.concatenate(cols_k, axis=1).reshape(8, 128, 128).astype(NPBF16)
            bqp[pr, :, 0] = np.concatenate(bq_c)
            bkp[pr, :, 0] = np.concatenate(bk_c)

        # wv: per chunk [128, 2(pr) x 2(hh) x 64]
        wv = np.zeros((8, 128, 256), NPBF16)
        for pr in range(2):
            for hh in range(2):
                h = h0 + 2 * pr + hh
                vcols = Wkv[:, h * 2 * DH + DH : (h + 1) * 2 * DH]  # [1024, 64]
                wv[:, :, (pr * 2 + hh) * 64 : (pr * 2 + hh + 1) * 64] = (
                    vcols.reshape(8, 128, 64).astype(NPBF16)
                )

        wo_rows = Wo[h0 * DH : (h0 + HPC) * DH, :]  # [256, 1024]
        bmask = (INF * (mask_kv[b].astype(np.float32) - 1.0)).reshape(NB, 128, 1)

        in_maps.append(
            {
                "sqt": np.ascontiguousarray(s_q[b].T).astype(NPBF16),
                "skvt": np.ascontiguousarray(s_kv[b].T).astype(NPBF16),
                "wq": wq,
                "wk": wk,
                "wq2": np.ascontiguousarray(wq2.reshape(8, 128, 128)).astype(NPBF16),
                "wk2": np.ascontiguousarray(wk2.reshape(8, 128, 128)).astype(NPBF16),
                "bq2": bq2,
                "bk2": bk2,
                "wv": wv,
                "wo": np.ascontiguousarray(wo_rows.reshape(2, 128, DOUT)).astype(NPBF16),
                "bq": bqp,
                "bk": bkp,
                "cost": cosT,
                "sint": sinT,
                "mtile": mtile2,
                "e64": e64,
                "bmask": bmask.astype(np.float32),
            }
        )
    return in_maps


_NC_CACHE = {}


def kernel(s_q, s_kv, mask_q, mask_kv, Wq, bq, Wkv, bkv, Wo, bo, _return_results=False):
    from concourse.bass_utils import run_bass_kernel_spmd

    if "nc" not in _NC_CACHE:
        _NC_CACHE["nc"] = build_nc()
    nc = _NC_CACHE["nc"]

    s_q = np.asarray(s_q, np.float32)
    s_kv = np.asarray(s_kv, np.float32)
    Wq_ = np.asarray(Wq, np.float32)
    Wkv_ = np.asarray(Wkv, np.float32)
    Wo_ = np.asarray(Wo, np.float32)
    bkv_ = np.asarray(bkv, np.float32)
    in_maps = _prep_host(
        s_q, s_kv,
        np.asarray(mask_q, np.float32),
        np.asarray(mask_kv, np.float32),
        Wq_, np.asarray(bq, np.float32), Wkv_, bkv_, Wo_, np.asarray(bo, np.float32),
    )
    trace = bool(int(os.environ.get("KERNEL_TRACE", "0")))
    res = run_bass_kernel_spmd(nc, in_maps, core_ids=list(range(8)), trace=trace)

    # v-bias contribution folded here: o_norm includes +bv per head, so add
    # bv_full @ Wo once per batch on the host (softmax weights sum to 1).
    bv_full = bkv_.reshape(H, 2 * DH)[:, DH:].reshape(-1)  # [H*DH]
    bo_eff = np.asarray(bo, np.float32) + bv_full @ Wo_

    out = np.zeros((B, N, DOUT), np.float32)
    for core in range(8):
        b = core // 4
        out[b] += res.results[core]["out"].astype(np.float32)
    out += bo_eff[None, None, :]
    if _return_results:
        return out, res
    return out
